# revision 27
# baseline (speedup 1.0000x reference)
"""Trainium2 Bass kernel for nn_DGM_c (DGM graph-construction layer).

Reference computation (see problem statement):
    x_emb = relu(A @ x @ W + b)                       [B,N,E]
    centroid = mean(x_emb, axis=-2); scale = 0.9/max|x_emb-centroid|
    xs = (x_emb-centroid)*scale
    D = cdist(xs)^2 ; adj = sigmoid(T*(|thr| - D))
    edge_index = fixed arange pattern ; edges_weight = adj.reshape(-1)

Key identities used:
  * centroid cancels in pairwise differences:
        D_ij = scale^2 * (|e_i|^2 + |e_j|^2 - 2 e_i.e_j),  e = x_emb,
    so only one global scalar (scale) couples all rows -> two SPMD
    launches over 8 NeuronCores (core c -> batch c//4, row-block c%4 of
    512 rows) with a tiny host step between them.
  * associativity: A@x@W == A@(x@W); x@W is tiny and computed on host,
    so the device only runs the N x N x E contraction.

All matmuls use a bf16 hi/lo split (a = hi+lo; passes hi*hi + hi*lo +
lo*hi) — the PE runs at full bf16 rate (4x the fp32 mode) at ~1e-5
relative accuracy, and DMA bytes stay at fp32 size.  Phase 2 packs two
of the three passes into one matmul along the contraction axis
(K=64+64 and K=64+1+1), since matmul cost scales with output columns,
not K.

  Launch 1 (per core): e_blk^T = relu((xW)^T A_blk^T + b) via 3x16
    accumulated bf16 matmuls into one [64,512] PSUM tile; xW|A^T hi/lo
    packed in one DRAM tensor, streamed in 7 chunks sized so the first
    matmuls start early and the post-DMA compute tail is short.
  Host: assemble x_emb, compute scale/sq norms, fold every constant
    into the phase-2 operands.
  Launch 2 (per core): per [128,512] PSUM bank: K=128 matmul
    ([2s2*e_hi; 2s2*e_hi] x [e_hi; e_lo]) + K=66 matmul
    ([2s2*e_lo; 1; 1] x [e_hi; -s2*sq_hi; -s2*sq_lo]) give
    2*T*scale^2*G - T*scale^2*sq_j; ScalarE applies
    sigmoid(psum + (T|thr| - T*scale^2*sq_i)) over [128,1024] spans;
    outputs ship as two 0.5MiB DMAs for the first and last row-tiles
    (short pipeline lead-in; early queue drain at the tail) and one
    1MiB DMA for each middle row-tile; tiny bias DMAs ride the SWDGE
    queue to keep the HWDGE issue chain clear.

edge_index is input-independent -> generated host-side.
"""

import os
import sys
from contextlib import ExitStack

for _p in ("/opt/trn_rl_repo", "/root/.axon_site/_ro/trn_rl_repo"):
    if os.path.isdir(_p) and _p not in sys.path:
        sys.path.insert(0, _p)

import ml_dtypes
import numpy as np

import concourse.bass as bass  # noqa: F401  (registers engines)
import concourse.tile as tile
from concourse import bacc, mybir
from concourse.bass_utils import run_bass_kernel_spmd

B, N, F_IN, F_EMB = 2, 2048, 128, 64
NCORES = 8
CPB = NCORES // B          # cores per batch
R = N // CPB               # rows per core = 512
KT = N // 128              # contraction tiles = 16
IT = R // 128              # row tiles per core = 4
JT = N // 512              # column chunks per core = 4
C1 = F_EMB + R             # packed (xW | A^T) row width = 576
KSPLIT = (1, 2, 4, 3, 3, 2, 1)          # k-tiles per phase-1 chunk
F32 = mybir.dt.float32
BF16 = mybir.dt.bfloat16
NPBF16 = ml_dtypes.bfloat16
CORE_IDS = list(range(NCORES))

_NC_CACHE: dict = {}


def _split_hl(a: np.ndarray) -> tuple[np.ndarray, np.ndarray]:
    """fp32 -> (hi, lo) bf16 pair with hi + lo ~= a to ~2^-17 relative."""
    hi = a.astype(NPBF16)
    lo = (a - hi.astype(np.float32)).astype(NPBF16)
    return hi, lo


def _build_phase1():
    """e_blk^T = relu((xW)^T A_blk^T + b) for this core's 512 rows."""
    nc = bacc.Bacc("TRN2", target_bir_lowering=False, debug=False,
                   num_devices=NCORES)
    # pk[n] = [hi(xW[n] | A[b, rows, n]) | lo(...)]
    pk_ap = nc.dram_tensor("pk", [N, 2 * C1], BF16, kind="ExternalInput").ap()
    bb_ap = nc.dram_tensor("bb", [F_EMB, 1], F32, kind="ExternalInput").ap()
    et_ap = nc.dram_tensor("et", [F_EMB, R], F32, kind="ExternalOutput").ap()

    with tile.TileContext(nc) as tc, ExitStack() as ctx:
        const = ctx.enter_context(tc.tile_pool(name="const", bufs=1))
        pkp = ctx.enter_context(tc.tile_pool(name="pkp", bufs=4))
        pst = ctx.enter_context(tc.tile_pool(name="pst", bufs=1, space="PSUM"))
        spool = ctx.enter_context(tc.tile_pool(name="sp", bufs=2))

        pk_r = pk_ap.rearrange("(k p) c -> p k c", p=128)  # [128, KT, 2*C1]
        psum_t = pst.tile([F_EMB, R], F32)
        nmm = 3 * KT
        mmi = 0
        k0 = 0
        for nk in KSPLIT:
            ch = pkp.tile([128, nk * 2 * C1], BF16, tag="chunk")
            nc.sync.dma_start(
                ch[:].rearrange("p (k c) -> p k c", k=nk),
                pk_r[:, k0:k0 + nk, :],
            )
            for kk in range(nk):
                base = kk * 2 * C1
                w_hi = ch[:, base:base + F_EMB]                    # xW hi
                a_hi = ch[:, base + F_EMB:base + C1]               # A^T hi
                w_lo = ch[:, base + C1:base + C1 + F_EMB]          # xW lo
                a_lo = ch[:, base + C1 + F_EMB:base + 2 * C1]      # A^T lo
                for lhsT, rhs in ((w_hi, a_hi), (w_hi, a_lo), (w_lo, a_hi)):
                    nc.tensor.matmul(psum_t[:], lhsT, rhs,
                                     start=(mmi == 0), stop=(mmi == nmm - 1))
                    mmi += 1
            k0 += nk

        bb = const.tile([F_EMB, 1], F32)
        nc.gpsimd.dma_start(bb[:], bb_ap[:])   # SWDGE: keep HWDGE chain clear
        esb = spool.tile([F_EMB, R], F32)
        nc.scalar.activation(esb[:], psum_t[:],
                             mybir.ActivationFunctionType.Relu, bias=bb[:])
        nc.sync.dma_start(et_ap[:], esb[:])

    nc.compile()
    return nc


def _build_phase2():
    """w_blk = sigmoid(2*T*s^2*G - T*s^2*(sq_i+sq_j) + T|thr|) rows."""
    nc = bacc.Bacc("TRN2", target_bir_lowering=False, debug=False,
                   num_devices=NCORES)
    W2 = R + N                                          # 2560
    # pass-1 operands (K=128): pa[:, 0:R] = [lh_hi; lh_hi],
    #                          pa[:, R:]  = [rh_hi; rh_lo]
    pa_ap = nc.dram_tensor("pa", [128, W2], BF16, kind="ExternalInput").ap()
    # pass-2 operands (K=66):  pb[:, 0:R] = [lh_lo; 1; 1],
    #                          pb[:, R:]  = [rh_hi; aug_hi; aug_lo]
    pb_ap = nc.dram_tensor("pb", [F_EMB + 2, W2], BF16,
                           kind="ExternalInput").ap()
    bi_ap = nc.dram_tensor("bi", [128, IT], F32, kind="ExternalInput").ap()
    wo_ap = nc.dram_tensor("wo", [R, N], F32, kind="ExternalOutput").ap()

    with tile.TileContext(nc) as tc, ExitStack() as ctx:
        inp = ctx.enter_context(tc.tile_pool(name="inp", bufs=1))
        psp = ctx.enter_context(tc.tile_pool(name="psp", bufs=2, space="PSUM"))
        outp = ctx.enter_context(tc.tile_pool(name="outp", bufs=3))

        # input DMAs split so pass-1/pass-2 matmuls can start as soon as
        # their lhsT + first rhs half land (shorter pipeline lead-in)
        H = N // 2
        pal = inp.tile([128, R], BF16)
        nc.sync.dma_start(pal[:], pa_ap[:, 0:R])
        pbl = inp.tile([F_EMB + 2, R], BF16)
        nc.sync.dma_start(pbl[:], pb_ap[:, 0:R])
        bi = inp.tile([128, IT], F32)
        nc.gpsimd.dma_start(bi[:], bi_ap[:])   # SWDGE: keep HWDGE chain clear
        pars, pbrs = [], []
        for q in range(2):
            par = inp.tile([128, H], BF16, tag=f"par{q}")
            nc.sync.dma_start(par[:], pa_ap[:, R + q * H:R + (q + 1) * H])
            pars.append(par)
            pbr = inp.tile([F_EMB + 2, H], BF16, tag=f"pbr{q}")
            nc.sync.dma_start(pbr[:], pb_ap[:, R + q * H:R + (q + 1) * H])
            pbrs.append(pbr)

        wo_r = wo_ap.rearrange("(i p) n -> p i n", p=128)   # [128, IT, N]
        for i in range(IT):
            # i=0 ships each half as soon as its sigmoid lands (short lead);
            # later i's coalesce both halves into one 1MiB DMA (less per-DMA
            # overhead on the serialized DMA chain).  PSUM tiles are
            # allocated full-width but each (i, h) uses only slice h —
            # the staggered bank rotation schedules best.
            wfull = (None if i in (0, IT - 1)
                     else outp.tile([128, N], F32, tag="wf"))
            for h in range(2):                              # half row-tiles
                ps = psp.tile([128, N], F32, tag="ps")
                pss = ps[:, h * H:(h + 1) * H]
                for jj in range(JT // 2):
                    j = 2 * h + jj
                    out = pss[:, jj * 512:(jj + 1) * 512]
                    nc.tensor.matmul(out, pal[:, i * 128:(i + 1) * 128],
                                     pars[j // 2][:, (j % 2) * 512:
                                                   ((j % 2) + 1) * 512],
                                     start=True, stop=False)
                    nc.tensor.matmul(out, pbl[:, i * 128:(i + 1) * 128],
                                     pbrs[j // 2][:, (j % 2) * 512:
                                                  ((j % 2) + 1) * 512],
                                     start=False, stop=True)
                if wfull is None:
                    wsb = outp.tile([128, H], F32, tag="wsb")
                    nc.scalar.activation(wsb[:], pss[:],
                                         mybir.ActivationFunctionType.Sigmoid,
                                         bias=bi[:, i:i + 1])
                    nc.sync.dma_start(wo_r[:, i, h * H:(h + 1) * H], wsb[:])
                else:
                    nc.scalar.activation(
                        wfull[:, h * H:(h + 1) * H], pss[:],
                        mybir.ActivationFunctionType.Sigmoid,
                        bias=bi[:, i:i + 1])
            if wfull is not None:
                nc.sync.dma_start(wo_r[:, i, :], wfull[:])

    nc.compile()
    return nc


def _get_nc(key, builder):
    nc = _NC_CACHE.get(key)
    if nc is None:
        nc = builder()
        _NC_CACHE[key] = nc
    return nc


def _run_spmd(nc, in_maps):
    """run_bass_kernel_spmd with retries — the axon-tunneled device
    occasionally reports a transient NRT_EXEC_UNIT_UNRECOVERABLE on the
    first execution of a freshly compiled NEFF; a re-run succeeds."""
    last = None
    for _ in range(3):
        try:
            return run_bass_kernel_spmd(nc, in_maps, core_ids=CORE_IDS)
        except Exception as e:  # noqa: BLE001
            last = e
    raise last


def _edge_index() -> np.ndarray:
    idx = np.arange(B * N * N, dtype=np.int32)
    rows = idx // N
    cols = idx % N + N * (rows // N)
    return np.stack([rows, cols]).astype(np.int32)


def kernel(x, A, W_embed, b_embed, temperature, threshold):
    x = np.asarray(x, dtype=np.float32)
    A = np.asarray(A, dtype=np.float32)
    W_embed = np.asarray(W_embed, dtype=np.float32)
    b_embed = np.asarray(b_embed, dtype=np.float32)
    T = np.float32(np.asarray(temperature).reshape(()))
    thr = np.abs(np.float32(np.asarray(threshold).reshape(())))

    # ---- launch 1: x_emb ----
    nc1 = _get_nc("p1", _build_phase1)
    xW = x @ W_embed                                     # [B, N, F_EMB] fp32
    bb = b_embed.reshape(F_EMB, 1)
    in1 = []
    for c in range(NCORES):
        b, rb = divmod(c, CPB)
        pkf = np.empty((N, C1), dtype=np.float32)
        pkf[:, :F_EMB] = xW[b]
        pkf[:, F_EMB:] = A[b, rb * R:(rb + 1) * R, :].T
        hi, lo = _split_hl(pkf)
        pk = np.concatenate([hi, lo], axis=1)            # [N, 2*C1]
        in1.append({"pk": pk, "bb": bb})
    res1 = _run_spmd(nc1, in1)

    x_emb = np.empty((B, N, F_EMB), dtype=np.float32)
    for c in range(NCORES):
        b, rb = divmod(c, CPB)
        x_emb[b, rb * R:(rb + 1) * R, :] = res1.results[c]["et"].T

    # ---- host: global scale + fold constants ----
    centroid = x_emb.mean(axis=1, keepdims=True, dtype=np.float32)
    scale = np.float32(0.9) / np.abs(x_emb - centroid).max()
    s2 = np.float32(T * scale * scale)          # T * scale^2
    sq0 = np.einsum("bne,bne->bn", x_emb, x_emb).astype(np.float32)  # [B,N]

    nc2 = _get_nc("p2", _build_phase2)
    in2 = []
    for c in range(NCORES):
        b, rb = divmod(c, CPB)
        eT = x_emb[b].T                          # [E, N] fp32
        e_hi, e_lo = _split_hl(eT)
        lh_hi, lh_lo = _split_hl((2.0 * s2) * eT[:, rb * R:(rb + 1) * R])
        a_hi, a_lo = _split_hl((-s2) * sq0[b])

        pa = np.empty((128, R + N), dtype=NPBF16)
        pa[:F_EMB, :R] = lh_hi
        pa[F_EMB:, :R] = lh_hi
        pa[:F_EMB, R:] = e_hi
        pa[F_EMB:, R:] = e_lo

        pb = np.empty((F_EMB + 2, R + N), dtype=NPBF16)
        pb[:F_EMB, :R] = lh_lo
        pb[F_EMB:, :R] = NPBF16(1.0)
        pb[:F_EMB, R:] = e_hi
        pb[F_EMB, R:] = a_hi
        pb[F_EMB + 1, R:] = a_lo

        bi = (T * thr - s2 * sq0[b, rb * R:(rb + 1) * R])
        bi = np.ascontiguousarray(bi.reshape(IT, 128).T)   # [128, IT]
        in2.append({"pa": pa, "pb": pb, "bi": bi})
    res2 = _run_spmd(nc2, in2)

    adj = np.empty((B, N, N), dtype=np.float32)
    for c in range(NCORES):
        b, rb = divmod(c, CPB)
        adj[b, rb * R:(rb + 1) * R, :] = res2.results[c]["wo"]

    return x_emb, _edge_index(), adj.reshape(-1)
